# revision 12
# baseline (speedup 1.0000x reference)
"""Trainium2 8-core kernel for modality-routed attention (nn_Attention_21715354648747).

Strategy (per sharding hint + modality-sorted routing):
- Host: sort tokens by modality; fold pre_norm into qkv_w; build x^T bf16 block
  layout (replicated per core) so no activation AllGather is needed: the
  pre-norm 1/rms(x) scalar commutes through the QKV matmul and cancels in the
  q/k head RMS-norms; only v and the gate logits need it, so just the [1,256]
  per-core inv rows are AllGathered (1KB). Fold q/k_norm+rope into per-token
  tables; pre-tile weights per core (column-split QKV: 5 Q heads + 1 KV group
  + gates per core; row-split proj over the core's 640 head dims).
- Device: per-shard inv compute + tiny AllGather overlapped with routed QKV
  (weights stationary, per-modality contiguous token segments) -> QK norm +
  rope in [d,t] layout -> attention per head (scores^T, exp, PV matmul;
  softmax denominator via vector-accumulated exp tiles + one ones-matmul) ->
  gated, 1/denom scaled -> routed proj in five 1024-col chunks, each followed
  by its ReduceScatter so only the last chunk's RS is exposed.
- Host: concat shards, invert permutation, cast f32.
"""
import sys

for _p in ("/opt/trn_rl_repo",):
    if _p not in sys.path:
        sys.path.append(_p)

import numpy as np
import ml_dtypes

import concourse.bass as bass
import concourse.tile as tile
from concourse import mybir
from concourse.masks import make_identity

# ---------------- problem constants (hardcoded) ----------------
S = 2048
H = 5120
HB = H // 128          # 40 h-blocks
D = 128
NCORE = 8
SLOC = S // NCORE      # 256 tokens per core
NQH = 40
NHL = NQH // NCORE     # 5 q heads per core
NKV = 8
M = 3
QKV_PER_MOD = NQH * D + 2 * NKV * D + NQH  # 7208
Q_DIM = NQH * D        # 5120
K_DIM = NKV * D        # 1024
EPS = 1e-6
NCH = 5                # proj output chunks (1024 cols each)
CW = 2 * 512           # chunk width

BF = mybir.dt.bfloat16
F32 = mybir.dt.float32
AF = mybir.ActivationFunctionType
P = 128

_MAX_WAITS = 1
_wsplit_counter = [0]


def _split_excess_waits(nc, max_waits=_MAX_WAITS):
    """This walrus build encodes at most one sync wait per instruction; Tile's
    wait coalescing (notably the kernel-tail Drain) can exceed that. Move the
    excess waits onto NOPs inserted immediately before, on the same engine."""
    for fn in nc.m.functions:
        for bb in fn.blocks:
            il = bb.instructions
            snapshot = list(il)
            new_list = []
            changed = False
            for ins in snapshot:
                si = ins.sync_info
                waits = list(si.on_wait) if si is not None else []
                if len(waits) > max_waits:
                    extra = waits[: len(waits) - max_waits]
                    keep = waits[len(waits) - max_waits:]
                    for c in range(0, len(extra), max_waits):
                        chunk = extra[c:c + max_waits]
                        _wsplit_counter[0] += 1
                        nop = mybir.InstNoOp(
                            name=f"Wsplit-{_wsplit_counter[0]}", ins=[], outs=[]
                        )
                        nop.engine = ins.engine
                        nop.sync_info = mybir.SyncInfo(on_wait=chunk, on_update=[])
                        new_list.append(nop)
                        changed = True
                    si.on_wait[:] = keep
                new_list.append(ins)
            if changed:
                il[:] = new_list


def build_module(counts):
    """Build the SPMD Bass module for given modality counts (token-sorted)."""
    c0, c1, c2 = counts
    off = [0, c0, c0 + c1, S]
    nc = bass.Bass()

    # ---- DRAM parameters (per-core shards via in_maps) ----
    xs_ext = nc.declare_dram_parameter("xs", [SLOC, H], BF, isOutput=False)
    xt_ext = nc.declare_dram_parameter("xt", [NCORE, P, HB, SLOC], BF, isOutput=False)
    qkvw_ext = nc.declare_dram_parameter("qkvw", [M, 7, P, HB, P], BF, isOutput=False)
    gatew_ext = nc.declare_dram_parameter("gatew", [M, P, HB, 8], BF, isOutput=False)
    projw_ext = nc.declare_dram_parameter("projw", [M, 2 * NCH, P, NHL, 512], BF,
                                          isOutput=False)
    ropes_ext = nc.declare_dram_parameter("ropes", [4, P, S], BF, isOutput=False)
    bmask_ext = nc.declare_dram_parameter("bmask", [4, P, 1], F32, isOutput=False)
    out_ext = nc.declare_dram_parameter("out", [NCH, SLOC, CW], BF, isOutput=True)

    # ---- internal DRAM ----
    iag_in = nc.dram_tensor("iagin", [1, SLOC], BF)
    iag_out = nc.dram_tensor("iagout", [NCORE, 1, SLOC], BF, addr_space="Shared")
    gsd = nc.dram_tensor("gsd", [NHL, S], BF)
    yh = [nc.dram_tensor(f"y{i}", [S, CW], BF) for i in range(NCH)]
    rsh = [nc.dram_tensor(f"rs{i}", [SLOC, CW], BF) for i in range(NCH)]

    RG = [list(range(NCORE))]

    # boundary chunks for proj (tokens on partitions, chunks of 128)
    bnds = {}  # tc -> boundary idx (0: between mod0/1, 1: between mod1/2)
    for b in (1, 2):
        if off[b] % P != 0:
            bnds[off[b] // P] = b - 1

    # per-mod QKV matmul chunks: (token t0, width, nblk) with nblk=2 only for
    # 256-aligned full double-blocks (pair access pattern, N=512)
    def qkv_chunks(m):
        lo, hi = off[m], off[m + 1]
        out = []
        t = lo
        while t < hi:
            if t % SLOC != 0:
                nxt = min(hi, (t // SLOC + 1) * SLOC)
                out.append((t, nxt - t, 1))
            elif t + 2 * SLOC <= hi:
                nxt = t + 2 * SLOC
                out.append((t, 2 * SLOC, 2))
            else:
                nxt = min(hi, t + SLOC)
                out.append((t, nxt - t, 1))
            t = nxt
        return out

    with tile.TileContext(nc) as tc:
        with tc.tile_pool(name="const", bufs=1) as constp, \
             tc.tile_pool(name="resident", bufs=1) as resp:
            identb = constp.tile([P, P], BF)
            make_identity(nc, identb[:])
            ones_b = constp.tile([P, 1], BF)
            nc.vector.memset(ones_b[:], 1.0)

            # qkvT resident tiles: 0-4 q heads, 5 k, 6 v, 7 gates (then v_nat)
            qkvT = [resp.tile([P, S], BF, tag=f"qkvT{i}", name=f"qkvT{i}")
                    for i in range(8)]
            v_nat = qkvT[7]   # reused after gates move to DRAM
            ogt = qkvT[:NHL]  # roped q overwritten by gated attn out slices

            # x^T block staging (resident so loads can be issued first)
            BLKW = HB * SLOC
            xall = resp.tile([P, 4 * BLKW], BF, tag="xall", name="xall")
            slot_of = {}   # block -> slot

            def load_xblks(m, first=None):
                lo, hi = off[m], off[m + 1]
                r0, r1 = lo // SLOC, (hi - 1) // SLOC
                need = list(range(r0, r1 + 1))
                if first is not None:
                    need = need[:first]
                else:
                    # evict blocks not needed; keep shared ones
                    for b in list(slot_of):
                        if b not in need:
                            del slot_of[b]
                free = sorted(set(range(4)) - set(slot_of.values()))
                for r in need:
                    if r in slot_of:
                        continue
                    sl = free.pop(0)
                    slot_of[r] = sl
                    nc.sync.dma_start(
                        xall[:, sl * BLKW:(sl + 1) * BLKW].rearrange(
                            "p (hb f) -> p hb f", hb=HB),
                        xt_ext[r])
                return None

            load_xblks(0, first=2)  # first two mod0 blocks ahead of all else

            ropes = constp.tile([P, 4 * S], BF)
            bmask = constp.tile([P, 4], F32)

            def load_consts():  # deferred: keep startup DMA for x blocks
                nc.sync.dma_start(
                    ropes[:].rearrange("p (a f) -> p a f", a=4),
                    ropes_ext.rearrange("a p f -> p a f"))
                nc.sync.dma_start(
                    bmask[:].rearrange("p (a f) -> p a f", a=4),
                    bmask_ext.rearrange("a p f -> p a f"))

            pbw_cm = tc.tile_pool(name="phB_w", bufs=2)
            pbw = pbw_cm.__enter__()

            def load_gatew(m):
                gwb = pbw.tile([P, HB * 8], BF, tag="gwb", name="gwb")
                nc.sync.dma_start(
                    gwb[:], gatew_ext[m].rearrange("p hb g -> p (hb g)"))
                return gwb

            gwb0 = load_gatew(0)   # tiny first-weight tile: first MM needs
            load_xblks(0)          # only blocks 0-1 + 51KB of gate weights

            # ---------------- phase A: per-shard inv(rms) + tiny AllGather ----
            with tc.tile_pool(name="phA", bufs=2) as pa:
                for tt in range(SLOC // P):
                    xtf = pa.tile([P, H], BF, tag="xt")
                    nc.sync.dma_start(xtf[:], xs_ext[tt * P:(tt + 1) * P, :])
                    sq = pa.tile([P, H], BF, tag="sq")
                    ssq = pa.tile([P, 1], F32, tag="ssq")
                    nc.scalar.activation(sq[:], xtf[:], AF.Square, accum_out=ssq[:])
                    z = pa.tile([P, 1], F32, tag="z")
                    nc.vector.tensor_scalar(z[:], ssq[:], 1.0 / H, EPS,
                                            mybir.AluOpType.mult, mybir.AluOpType.add)
                    zr = pa.tile([P, 1], F32, tag="zr")
                    nc.vector.reciprocal(zr[:], z[:])
                    inv = pa.tile([P, 1], BF, tag="inv")
                    nc.scalar.activation(inv[:], zr[:], AF.Sqrt)
                    nc.sync.dma_start(
                        iag_in[0:1, tt * P:(tt + 1) * P].rearrange("o t -> t o"),
                        inv[:])
            nc.gpsimd.collective_compute(
                "AllGather", mybir.AluOpType.bypass, replica_groups=RG,
                ins=[iag_in[:]], outs=[iag_out[:]])

            # ------- phases B/C/D interleaved: QKV, norms+rope, attention ----
            OT_ORDER = [7, 6, 5, 0, 1, 2, 3, 4]
            with tc.tile_pool(name="phC", bufs=1) as pc, \
                 tc.tile_pool(name="phCg", bufs=1) as pcg, \
                 tc.tile_pool(name="phD", bufs=2) as pd, \
                 tc.tile_pool(name="phD_es", bufs=4) as pes, \
                 tc.tile_pool(name="phB_ps", bufs=2, space="PSUM") as pbps, \
                 tc.tile_pool(name="phC_ps", bufs=1, space="PSUM") as pcps, \
                 tc.tile_pool(name="phD_s", bufs=3, space="PSUM") as pds, \
                 tc.tile_pool(name="phD_o", bufs=2, space="PSUM") as pdo, \
                 tc.tile_pool(name="dramp", bufs=3, space="DRAM") as drp:

                def phase_b(m, ot, wb=None):
                    chunks = qkv_chunks(m)
                    gate = ot == 7
                    if wb is not None:
                        wbuf = wb
                    elif gate:
                        wbuf = load_gatew(m)
                    else:
                        wbuf = pbw.tile([P, HB * P], BF, tag="wbuf", name="wbuf")
                        nc.sync.dma_start(
                            wbuf[:].rearrange("p (hb f) -> p hb f", hb=HB),
                            qkvw_ext[m, ot])
                    nout = NHL if gate else P
                    wstride = 8 if gate else P
                    x4 = xall[:].rearrange("p (b hb f) -> p b hb f", b=4, hb=HB)
                    # resolve chunks to slots; split pairs whose slots are not
                    # adjacent in the ring
                    rchunks = []
                    for (t0, w, nblk) in chunks:
                        b = t0 // SLOC
                        if nblk == 2 and slot_of[b + 1] != slot_of[b] + 1:
                            rchunks.append((slot_of[b], 0, SLOC, 1, t0))
                            rchunks.append((slot_of[b + 1], 0, SLOC, 1, t0 + SLOC))
                        else:
                            rchunks.append((slot_of[b], t0 % SLOC, w, nblk, t0))
                    # chunk-major: each chunk finishes (and frees its x
                    # block) before later chunks; copies drain per chunk
                    for (sl, cl, w, nblk, t0) in rchunks:
                        ps = pbps.tile([P, 512], F32, tag="qkvps", name="qkvps")
                        for hb in range(HB):
                            if nblk == 2:
                                rhs = x4[:, sl:sl + 2, hb, :]
                            else:
                                rhs = x4[:, sl, hb, cl:cl + w]
                            nc.tensor.matmul(
                                ps[:nout, :w],
                                wbuf[:, hb * wstride:hb * wstride + nout], rhs,
                                start=(hb == 0), stop=(hb == HB - 1))
                        nc.vector.tensor_copy(
                            qkvT[ot][:nout, t0:t0 + w], ps[:nout, :w])

                def phase_c(kk):
                    src = qkvT[kk]
                    is_q = kk < NHL
                    sq = pc.tile([P, S], BF, tag="csq", name="csq")
                    nc.scalar.activation(sq[:], src[:], AF.Square)
                    invrow = pc.tile([1, S], BF, tag="invrow", name="invrow")
                    for ic in range(S // 512):
                        ssp = pcps.tile([1, 512], F32, tag="cps", name="ssp")
                        nc.tensor.matmul(ssp[:], ones_b[:],
                                         sq[:, ic * 512:(ic + 1) * 512],
                                         start=True, stop=True)
                        z = pc.tile([1, 512], F32, tag="cz", name="cz")
                        if is_q:
                            # fold 1/sqrt(D): rsqrt(ssq + D*eps)
                            nc.vector.tensor_scalar_add(z[:], ssp[:], D * EPS)
                        else:
                            nc.vector.tensor_scalar(z[:], ssp[:], 1.0 / D, EPS,
                                                    mybir.AluOpType.mult,
                                                    mybir.AluOpType.add)
                        zr = pc.tile([1, 512], F32, tag="czr", name="czr")
                        nc.vector.reciprocal(zr[:], z[:])
                        nc.scalar.activation(invrow[:, ic * 512:(ic + 1) * 512],
                                             zr[:], AF.Sqrt)
                    invdt = drp.tile([1, S], BF, tag="invd", name="invdt")
                    nc.sync.dma_start(invdt[:], invrow[:])
                    invb = pc.tile([P, S], BF, tag="invb", name="invb")
                    nc.sync.dma_start(invb[:], invdt[0:1, :].to_broadcast([P, S]))
                    sh = pc.tile([P, S], BF, tag="csh", name="csh")
                    nc.sync.dma_start(sh[0:64, :], src[64:128, :])
                    nc.sync.dma_start(sh[64:128, :], src[0:64, :])
                    A = ropes[:, (0 if is_q else 2) * S:(1 if is_q else 3) * S]
                    B = ropes[:, (1 if is_q else 3) * S:(2 if is_q else 4) * S]
                    t1 = pc.tile([P, S], BF, tag="ct1", name="ct1")
                    nc.vector.tensor_mul(t1[:], src[:], A)
                    t2 = pc.tile([P, S], BF, tag="ct2", name="ct2")
                    nc.vector.tensor_mul(t2[:], sh[:], B)
                    nc.vector.tensor_add(t1[:], t1[:], t2[:])
                    nc.vector.tensor_mul(src[:], t1[:], invb[:])  # roped in place

                def phase_d(hh):
                    rk = qkvT[NHL]
                    grow = pcg.tile([1, S], BF, tag="grow", name="grow")
                    nc.sync.dma_start(grow[:], gsd[hh:hh + 1, :])

                    def fin1(isl, esum):
                        # denominator matmul + gate/denominator row; deferred
                        # past the next ic's first j so the PE never waits on
                        # the esum vector chain
                        pden = pcps.tile([1, 512], F32, tag="cps", name="pden")
                        nc.tensor.matmul(pden[:], ones_b[:], esum[:],
                                         start=True, stop=True)
                        rden = pd.tile([1, 512], F32, tag="rden", name="rden")
                        nc.vector.reciprocal(rden[:], pden[:])
                        frow = pd.tile([1, 512], BF, tag="frow", name="frow")
                        nc.vector.tensor_mul(frow[:], rden[:], grow[0:1, isl])
                        facd = drp.tile([1, 512], BF, tag="facd", name="facd")
                        nc.sync.dma_start(facd[:], frow[:])
                        facb = pd.tile([P, 512], BF, tag="facb", name="facb")
                        nc.sync.dma_start(facb[:],
                                          facd[0:1, :].to_broadcast([P, 512]))
                        return facb

                    def fin2(isl, po, facb):
                        oev = pd.tile([P, 512], BF, tag="oev", name="oev")
                        nc.vector.tensor_copy(oev[:], po[:])
                        # qkvT[hh][:, isl] (roped q) is dead after its j-loop
                        nc.vector.tensor_mul(ogt[hh][:, isl], oev[:], facb[:])

                    pend = None
                    for ic in range(S // 512):
                        isl = slice(ic * 512, (ic + 1) * 512)
                        po = pdo.tile([P, 512], F32, tag="po", name="po")
                        esum = pd.tile([P, 512], BF, tag="esum", name="esum")
                        for j in range(S // P):
                            psc = pds.tile([P, 512], F32, tag="psc", name="psc")
                            nc.tensor.matmul(psc[:], rk[:, j * P:(j + 1) * P],
                                             qkvT[hh][:, isl], start=True, stop=True)
                            es = pes.tile([P, 512], BF, tag="es", name="es")
                            nc.scalar.activation(es[:], psc[:], AF.Exp)
                            nc.tensor.matmul(po[:], v_nat[:, j * P:(j + 1) * P], es[:],
                                             start=(j == 0), stop=(j == S // P - 1))
                            if j == 0:
                                nc.gpsimd.tensor_copy(esum[:], es[:])
                                if pend is not None:
                                    pisl, ppo, pesum = pend
                                    pfacb = fin1(pisl, pesum)
                                    pend = (pisl, ppo, pfacb)
                            else:
                                nc.gpsimd.tensor_add(esum[:], esum[:], es[:])
                        if pend is not None:
                            pisl, ppo, pfacb = pend
                            fin2(pisl, ppo, pfacb)
                        pend = (isl, po, esum)
                    pisl, ppo, pesum = pend
                    pfacb = fin1(pisl, pesum)
                    fin2(pisl, ppo, pfacb)

                for m in range(2):
                    load_xblks(m)
                    for ot in OT_ORDER:
                        phase_b(m, ot, wb=gwb0 if (m == 0 and ot == 7) else None)
                    if m == 0:
                        load_consts()
                load_xblks(2)
                for ot in OT_ORDER[:3]:
                    phase_b(2, ot)
                    if ot == 7:
                        # gate logits need the pre-norm scalar: g*inv, sigmoid
                        invb5 = pc.tile([NHL, S], BF, tag="invb5", name="invb5")
                        nc.sync.dma_start(
                            invb5[:],
                            iag_out.rearrange("r o t -> o (r t)")[0:1, :]
                            .to_broadcast([NHL, S]))
                        nc.vector.tensor_mul(qkvT[7][0:NHL, :],
                                             qkvT[7][0:NHL, :], invb5[:])
                        grows = pc.tile([NHL, S], BF, tag="grows", name="grows")
                        nc.scalar.activation(grows[:], qkvT[7][0:NHL, :],
                                             AF.Sigmoid)
                        nc.sync.dma_start(gsd[:], grows[:])
                    elif ot == 6:
                        pass  # v ready; transposed after gates leave qkvT[7]
                    elif ot == 5:
                        # inv columns for v scaling: [t_local, block] layout
                        invsbh = pc.tile([P, S // P], BF, tag="invsbh",
                                         name="invsbh")
                        nc.sync.dma_start(
                            invsbh[:].rearrange("p (r b) -> p r b", r=NCORE),
                            iag_out.rearrange("r o (b p) -> p (r o) b", b=2))
                        invsb = pc.tile([P, S // P], F32, tag="invsb",
                                        name="invsb")
                        nc.scalar.activation(invsb[:], invsbh[:], AF.Copy)
                        for j in range(S // P):
                            tp = pcps.tile([P, P], BF, tag="cps", name="vtp")
                            nc.tensor.transpose(tp[:], qkvT[6][:, j * P:(j + 1) * P],
                                                identb[:])
                            nc.vector.tensor_scalar_mul(
                                v_nat[:, j * P:(j + 1) * P], tp[:],
                                invsb[:, j:j + 1])
                        phase_c(NHL)  # k
                # q heads software-pipelined: B(h) -> C(h) with D(h-1)
                # emitted after C(h) so the PE queue never waits on the
                # rope chain of the head it is about to process
                for h in range(NHL):
                    phase_b(2, h)
                    phase_c(h)
                    if h > 0:
                        phase_d(h - 1)
                phase_d(NHL - 1)
            pbw_cm.__exit__(None, None, None)

            # ---------------- phase E: routed proj + chunked ReduceScatter ---
            with tc.tile_pool(name="phE_w", bufs=2) as pew, \
                 tc.tile_pool(name="phE", bufs=8) as pe, \
                 tc.tile_pool(name="phE_h", bufs=1) as peh, \
                 tc.tile_pool(name="phE_ps", bufs=4, space="PSUM") as peps:
                for ch in range(NCH):
                    holds = {}
                    for m in range(M):
                        lo, hi = off[m], off[m + 1]
                        pw = pew.tile([P, 2 * NHL * 512], BF, tag="pw", name="pw")
                        nc.sync.dma_start(
                            pw[:].rearrange("p (oc hb f) -> p oc hb f", oc=2, hb=NHL),
                            projw_ext[m, ch * 2:(ch + 1) * 2].rearrange(
                                "oc p hb f -> p oc hb f"))
                        tc0, tc1 = lo // P, (hi - 1) // P
                        for tcx in range(tc0, tc1 + 1):
                            ystage = pe.tile([P, CW], BF, tag="ystage",
                                             name="ystage")
                            for ol in range(2):
                                ps = peps.tile([P, 512], F32, tag="yps", name="yps")
                                for hb in range(NHL):
                                    nc.tensor.matmul(
                                        ps[:], ogt[hb][:, tcx * P:(tcx + 1) * P],
                                        pw[:, (ol * NHL + hb) * 512:
                                           (ol * NHL + hb + 1) * 512],
                                        start=(hb == 0), stop=(hb == NHL - 1))
                                nc.vector.tensor_copy(
                                    ystage[:, ol * 512:(ol + 1) * 512], ps[:])
                            if tcx in bnds:
                                bidx = bnds[tcx]
                                if m == bidx:  # lower mod: hold masked partial
                                    hv = peh.tile([P, CW], BF, tag=f"hold{bidx}",
                                                  name=f"hold{bidx}")
                                    nc.vector.tensor_scalar_mul(
                                        hv[:], ystage[:],
                                        bmask[:, 2 * bidx:2 * bidx + 1])
                                    holds[tcx] = hv
                                else:  # upper mod: merge with inverse mask
                                    hv = holds.pop(tcx)
                                    mg = pe.tile([P, CW], BF, tag="mg", name="mg")
                                    nc.vector.tensor_scalar_mul(
                                        mg[:], ystage[:],
                                        bmask[:, 2 * bidx + 1:2 * bidx + 2])
                                    yo = pe.tile([P, CW], BF, tag="yo", name="yo")
                                    nc.vector.tensor_add(yo[:], hv[:], mg[:])
                                    nc.sync.dma_start(
                                        yh[ch][tcx * P:(tcx + 1) * P, :], yo[:])
                            else:
                                nc.sync.dma_start(
                                    yh[ch][tcx * P:(tcx + 1) * P, :], ystage[:])
                    nc.gpsimd.collective_compute(
                        "ReduceScatter", mybir.AluOpType.add, replica_groups=RG,
                        ins=[yh[ch][:]], outs=[rsh[ch][:]])
                    nc.sync.dma_start(out_ext[ch], rsh[ch][:])

    _split_excess_waits(nc)
    return nc


# ---------------- host-side prep ----------------

def _prep_shards(x, rope_cos, rope_sin, modality_ids, pre_norm, qkv_w, q_norm,
                 k_norm, proj_w, perm, counts):
    """Build the 8 per-core in_maps (host work is index/layout prep only)."""
    bf16 = ml_dtypes.bfloat16
    mods = np.asarray(modality_ids).reshape(S)
    x2 = np.asarray(x).reshape(S, H)[perm]                      # sorted tokens
    cos = np.asarray(rope_cos).reshape(S, D)[perm]
    sin = np.asarray(rope_sin).reshape(S, D)[perm]
    mods_s = mods[perm]
    pn = np.asarray(pre_norm).reshape(M, H)
    qn = np.asarray(q_norm).reshape(M, D)
    kn = np.asarray(k_norm).reshape(M, D)
    qkv3 = np.asarray(qkv_w).reshape(M, QKV_PER_MOD, H)
    proj3 = np.asarray(proj_w).reshape(M, H, H)

    # x^T block layout [block, d, hb, t_local], bf16, replicated per core
    xt = np.ascontiguousarray(
        x2.reshape(NCORE, SLOC, HB, P).transpose(0, 3, 2, 1)).astype(bf16)

    # rope tables [d, t]: roped = q*A + shift64(q)*B   (norm weights folded in)
    def rope_tables(nw_tok):
        A = nw_tok * cos                                        # [S, D]
        Bm = np.empty_like(A)
        nw_sh = np.concatenate([nw_tok[:, 64:], nw_tok[:, :64]], axis=1)
        Bm[:, :64] = -nw_sh[:, :64] * sin[:, :64]
        Bm[:, 64:] = nw_sh[:, 64:] * sin[:, 64:]
        return A.T.astype(bf16), Bm.T.astype(bf16)              # [D, S]

    Aq, Bq = rope_tables(qn[mods_s])
    Ak, Bk = rope_tables(kn[mods_s])
    ropes = np.ascontiguousarray(np.stack([Aq, Bq, Ak, Bk]))    # [4, 128, S]

    # boundary masks for proj chunks
    off = [0, counts[0], counts[0] + counts[1], S]
    bmask = np.zeros((4, P, 1), np.float32)
    for b in (1, 2):
        if off[b] % P != 0:
            tcx = off[b] // P
            toks = tcx * P + np.arange(P)
            bmask[2 * (b - 1), :, 0] = (toks < off[b]).astype(np.float32)
            bmask[2 * (b - 1) + 1, :, 0] = (toks >= off[b]).astype(np.float32)

    in_maps = []
    for c in range(NCORE):
        rq = qkv3[:, c * NHL * D:(c + 1) * NHL * D, :]          # [3, 640, H]
        rk = qkv3[:, Q_DIM + c * D:Q_DIM + (c + 1) * D, :]      # [3, 128, H]
        rv = qkv3[:, Q_DIM + K_DIM + c * D:Q_DIM + K_DIM + (c + 1) * D, :]
        rg = qkv3[:, Q_DIM + 2 * K_DIM + c * NHL:Q_DIM + 2 * K_DIM + (c + 1) * NHL, :]
        Wm = np.concatenate([rq, rk, rv], axis=1)               # [3, 896, H]
        Wm = Wm * pn[:, None, :]                                # fold pre_norm
        WT = Wm.transpose(0, 2, 1)                              # [3, H, 896]
        qkvw = WT.reshape(M, HB, P, 7, P).transpose(0, 3, 2, 1, 4)
        qkvw = np.ascontiguousarray(qkvw).astype(bf16)          # [3,7,128,40,128]
        Gm = np.concatenate(
            [rg, np.zeros((M, 3, H), np.float32)], axis=1) * pn[:, None, :]
        gatew = Gm.transpose(0, 2, 1).reshape(M, HB, P, 8).transpose(0, 2, 1, 3)
        gatew = np.ascontiguousarray(gatew).astype(bf16)        # [3,128,40,8]

        PT = proj3[:, :, c * NHL * D:(c + 1) * NHL * D].transpose(0, 2, 1)
        projw = PT.reshape(M, NHL, P, 2 * NCH, 512).transpose(0, 3, 2, 1, 4)
        projw = np.ascontiguousarray(projw).astype(bf16)        # [3,10,128,5,512]

        in_maps.append({
            "xs": np.ascontiguousarray(x2[c * SLOC:(c + 1) * SLOC]).astype(bf16),
            "xt": xt,
            "qkvw": qkvw,
            "gatew": gatew,
            "projw": projw,
            "ropes": ropes,
            "bmask": bmask,
        })
    return in_maps


_CACHE = {}


def _get_module(counts):
    key = tuple(counts)
    if key not in _CACHE:
        _CACHE[key] = build_module(counts)
    return _CACHE[key]


def kernel(x, rope_cos, rope_sin, modality_ids, pre_norm, qkv_w, q_norm,
           k_norm, proj_w):
    mods = np.asarray(modality_ids).reshape(S)
    perm = np.argsort(mods, kind="stable")
    counts = [int((mods == m).sum()) for m in range(M)]
    assert min(counts) >= P, "modality segments must span at least one chunk"
    nc = _get_module(counts)
    in_maps = _prep_shards(x, rope_cos, rope_sin, modality_ids, pre_norm,
                           qkv_w, q_norm, k_norm, proj_w, perm, counts)
    res = run_bass_kernel_spmd(nc, in_maps, list(range(NCORE)), trace=False)
    y_sorted = np.empty((S, H), np.float32)
    for c in range(NCORE):
        o = res.results[c]["out"].astype(np.float32)            # [5, 256, 1024]
        for ch in range(NCH):
            y_sorted[c * SLOC:(c + 1) * SLOC, ch * CW:(ch + 1) * CW] = o[ch]
    y = np.empty_like(y_sorted)
    y[perm] = y_sorted
    return y[None]


from concourse.bass_utils import run_bass_kernel_spmd  # noqa: E402


# revision 15
# speedup vs baseline: 1.1218x; 1.1218x over previous
"""Trainium2 8-core kernel for modality-routed attention (nn_Attention_21715354648747).

Strategy (per sharding hint + modality-sorted routing):
- Host: sort tokens by modality; fold pre_norm into qkv_w; build x^T bf16 block
  layout (replicated per core) so no activation AllGather is needed: the
  pre-norm 1/rms(x) scalar commutes through the QKV matmul and cancels in the
  q/k head RMS-norms; only v and the gate logits need it, so just the [1,256]
  per-core inv rows are AllGathered (1KB). Fold q/k_norm+rope into per-token
  tables; pre-tile weights per core (column-split QKV: 5 Q heads + 1 KV group
  + gates per core; row-split proj over the core's 640 head dims).
- Device: per-shard inv compute + tiny AllGather overlapped with routed QKV
  (weights stationary, per-modality contiguous token segments) -> QK norm +
  rope in [d,t] layout -> attention per head (scores^T, exp, PV matmul;
  softmax denominator via vector-accumulated exp tiles + one ones-matmul) ->
  gated, 1/denom scaled -> routed proj in five 1024-col chunks, each followed
  by its ReduceScatter so only the last chunk's RS is exposed.
- Host: concat shards, invert permutation, cast f32.
"""
import sys

for _p in ("/opt/trn_rl_repo",):
    if _p not in sys.path:
        sys.path.append(_p)

import numpy as np
import ml_dtypes

import concourse.bass as bass
import concourse.tile as tile
from concourse import mybir
from concourse.masks import make_identity

# ---------------- problem constants (hardcoded) ----------------
S = 2048
H = 5120
HB = H // 128          # 40 h-blocks
D = 128
NCORE = 8
SLOC = S // NCORE      # 256 tokens per core
NQH = 40
NHL = NQH // NCORE     # 5 q heads per core
NKV = 8
M = 3
QKV_PER_MOD = NQH * D + 2 * NKV * D + NQH  # 7208
Q_DIM = NQH * D        # 5120
K_DIM = NKV * D        # 1024
EPS = 1e-6
# proj output chunks as (ol0, n_ol) over ten 512-col groups; small tail
# chunks so the final exposed ReduceScatter is short
ECH = [(0, 2), (2, 2), (4, 2), (6, 2), (8, 1), (9, 1)]
NCH = 10               # projw tiling stays ten 512-col groups
CW = 2 * 512           # max chunk width

BF = mybir.dt.bfloat16
F32 = mybir.dt.float32
AF = mybir.ActivationFunctionType
P = 128

_MAX_WAITS = 1
_wsplit_counter = [0]


def _split_excess_waits(nc, max_waits=_MAX_WAITS):
    """This walrus build encodes at most one sync wait per instruction; Tile's
    wait coalescing (notably the kernel-tail Drain) can exceed that. Move the
    excess waits onto NOPs inserted immediately before, on the same engine."""
    for fn in nc.m.functions:
        for bb in fn.blocks:
            il = bb.instructions
            snapshot = list(il)
            new_list = []
            changed = False
            for ins in snapshot:
                si = ins.sync_info
                waits = list(si.on_wait) if si is not None else []
                if len(waits) > max_waits:
                    extra = waits[: len(waits) - max_waits]
                    keep = waits[len(waits) - max_waits:]
                    for c in range(0, len(extra), max_waits):
                        chunk = extra[c:c + max_waits]
                        _wsplit_counter[0] += 1
                        nop = mybir.InstNoOp(
                            name=f"Wsplit-{_wsplit_counter[0]}", ins=[], outs=[]
                        )
                        nop.engine = ins.engine
                        nop.sync_info = mybir.SyncInfo(on_wait=chunk, on_update=[])
                        new_list.append(nop)
                        changed = True
                    si.on_wait[:] = keep
                new_list.append(ins)
            if changed:
                il[:] = new_list


def build_module(counts):
    """Build the SPMD Bass module for given modality counts (token-sorted)."""
    c0, c1, c2 = counts
    off = [0, c0, c0 + c1, S]
    nc = bass.Bass()

    # ---- DRAM parameters (per-core shards via in_maps) ----
    xs_ext = nc.declare_dram_parameter("xs", [SLOC, H], BF, isOutput=False)
    xt_ext = nc.declare_dram_parameter("xt", [NCORE, P, HB, SLOC], BF, isOutput=False)
    qkvw_ext = nc.declare_dram_parameter("qkvw", [M, 7, P, HB, P], BF, isOutput=False)
    gatew_ext = nc.declare_dram_parameter("gatew", [M, P, HB, 8], BF, isOutput=False)
    projw_ext = nc.declare_dram_parameter("projw", [M, NCH, P, NHL, 512], BF,
                                          isOutput=False)
    ropes_ext = nc.declare_dram_parameter("ropes", [4, P, S], BF, isOutput=False)
    bmask_ext = nc.declare_dram_parameter("bmask", [4, P, 1], F32, isOutput=False)
    out_ext = nc.declare_dram_parameter("out", [SLOC, H], BF, isOutput=True)

    # ---- internal DRAM ----
    iag_in = nc.dram_tensor("iagin", [1, SLOC], BF)
    iag_out = nc.dram_tensor("iagout", [NCORE, 1, SLOC], BF, addr_space="Shared")
    gsd = nc.dram_tensor("gsd", [NHL, S], BF)
    yh = [nc.dram_tensor(f"y{i}", [S, n * 512], BF)
          for i, (o, n) in enumerate(ECH)]
    rsh = [nc.dram_tensor(f"rs{i}", [SLOC, n * 512], BF)
           for i, (o, n) in enumerate(ECH)]

    RG = [list(range(NCORE))]

    # boundary chunks for proj (tokens on partitions, chunks of 128)
    bnds = {}  # tc -> boundary idx (0: between mod0/1, 1: between mod1/2)
    for b in (1, 2):
        if off[b] % P != 0:
            bnds[off[b] // P] = b - 1

    # per-mod QKV matmul chunks: (token t0, width, nblk) with nblk=2 only for
    # 256-aligned full double-blocks (pair access pattern, N=512)
    def qkv_chunks(m):
        lo, hi = off[m], off[m + 1]
        out = []
        t = lo
        while t < hi:
            if t % SLOC != 0:
                nxt = min(hi, (t // SLOC + 1) * SLOC)
                out.append((t, nxt - t, 1))
            elif t + 2 * SLOC <= hi:
                nxt = t + 2 * SLOC
                out.append((t, 2 * SLOC, 2))
            else:
                nxt = min(hi, t + SLOC)
                out.append((t, nxt - t, 1))
            t = nxt
        return out

    with tile.TileContext(nc) as tc:
        with tc.tile_pool(name="const", bufs=1) as constp, \
             tc.tile_pool(name="resident", bufs=1) as resp:
            identb = constp.tile([P, P], BF)
            make_identity(nc, identb[:])
            ones_b = constp.tile([P, 1], BF)
            nc.vector.memset(ones_b[:], 1.0)

            # qkvT resident tiles: 0-4 q heads, 5 k, 6 v, 7 gates (then v_nat)
            qkvT = [resp.tile([P, S], BF, tag=f"qkvT{i}", name=f"qkvT{i}")
                    for i in range(8)]
            v_nat = qkvT[7]   # reused after gates move to DRAM
            ogt = qkvT[:NHL]  # roped q overwritten by gated attn out slices

            # x^T block staging (resident so loads can be issued first)
            BLKW = HB * SLOC
            xall = resp.tile([P, 4 * BLKW], BF, tag="xall", name="xall")
            slot_of = {}   # block -> slot

            def load_xblks(m, first=None):
                lo, hi = off[m], off[m + 1]
                r0, r1 = lo // SLOC, (hi - 1) // SLOC
                need = list(range(r0, r1 + 1))
                if first is not None:
                    need = need[:first]
                else:
                    # evict blocks not needed; keep shared ones
                    for b in list(slot_of):
                        if b not in need:
                            del slot_of[b]
                free = sorted(set(range(4)) - set(slot_of.values()))
                for r in need:
                    if r in slot_of:
                        continue
                    sl = free.pop(0)
                    slot_of[r] = sl
                    nc.sync.dma_start(
                        xall[:, sl * BLKW:(sl + 1) * BLKW].rearrange(
                            "p (hb f) -> p hb f", hb=HB),
                        xt_ext[r])
                return None

            load_xblks(0, first=2)  # first two mod0 blocks ahead of all else

            ropes = constp.tile([P, 4 * S], BF)
            bmask = constp.tile([P, 4], F32)

            def load_consts():  # deferred: keep startup DMA for x blocks
                nc.sync.dma_start(
                    ropes[:].rearrange("p (a f) -> p a f", a=4),
                    ropes_ext.rearrange("a p f -> p a f"))
                nc.sync.dma_start(
                    bmask[:].rearrange("p (a f) -> p a f", a=4),
                    bmask_ext.rearrange("a p f -> p a f"))

            pbw_cm = tc.tile_pool(name="phB_w", bufs=2)
            pbw = pbw_cm.__enter__()

            def load_gatew(m):
                gwb = pbw.tile([P, HB * 8], BF, tag="gwb", name="gwb")
                nc.sync.dma_start(
                    gwb[:], gatew_ext[m].rearrange("p hb g -> p (hb g)"))
                return gwb

            def load_wbuf(m, ot):
                wbuf = pbw.tile([P, HB * P], BF, tag="wbuf", name="wbuf")
                nc.sync.dma_start(
                    wbuf[:].rearrange("p (hb f) -> p hb f", hb=HB),
                    qkvw_ext[m, ot])
                return wbuf

            wpre = {(0, 7): load_gatew(0)}  # tiny first-weight tile
            load_xblks(0)          # blocks 0-1 + 51KB gates ahead of the rest
            wpre[(0, 6)] = load_wbuf(0, 6)
            wpre[(0, 5)] = load_wbuf(0, 5)

            # ---------------- phase A: per-shard inv(rms) + tiny AllGather ----
            with tc.tile_pool(name="phA", bufs=2) as pa:
                for tt in range(SLOC // P):
                    xtf = pa.tile([P, H], BF, tag="xt")
                    nc.sync.dma_start(xtf[:], xs_ext[tt * P:(tt + 1) * P, :])
                    sq = pa.tile([P, H], BF, tag="sq")
                    ssq = pa.tile([P, 1], F32, tag="ssq")
                    nc.scalar.activation(sq[:], xtf[:], AF.Square, accum_out=ssq[:])
                    z = pa.tile([P, 1], F32, tag="z")
                    nc.vector.tensor_scalar(z[:], ssq[:], 1.0 / H, EPS,
                                            mybir.AluOpType.mult, mybir.AluOpType.add)
                    zr = pa.tile([P, 1], F32, tag="zr")
                    nc.vector.reciprocal(zr[:], z[:])
                    inv = pa.tile([P, 1], BF, tag="inv")
                    nc.scalar.activation(inv[:], zr[:], AF.Sqrt)
                    nc.sync.dma_start(
                        iag_in[0:1, tt * P:(tt + 1) * P].rearrange("o t -> t o"),
                        inv[:])
            nc.gpsimd.collective_compute(
                "AllGather", mybir.AluOpType.bypass, replica_groups=RG,
                ins=[iag_in[:]], outs=[iag_out[:]])

            # ------- phases B/C/D interleaved: QKV, norms+rope, attention ----
            OT_ORDER = [7, 6, 5, 0, 1, 2, 3, 4]
            with tc.tile_pool(name="phC", bufs=1) as pc, \
                 tc.tile_pool(name="phCg", bufs=1) as pcg, \
                 tc.tile_pool(name="phD", bufs=2) as pd, \
                 tc.tile_pool(name="phD_es", bufs=4) as pes, \
                 tc.tile_pool(name="phB_ps", bufs=2, space="PSUM") as pbps, \
                 tc.tile_pool(name="phC_ps", bufs=1, space="PSUM") as pcps, \
                 tc.tile_pool(name="phD_s", bufs=3, space="PSUM") as pds, \
                 tc.tile_pool(name="phD_o", bufs=2, space="PSUM") as pdo, \
                 tc.tile_pool(name="dramp", bufs=3, space="DRAM") as drp:

                def phase_b(m, ot, wb=None):
                    chunks = qkv_chunks(m)
                    gate = ot == 7
                    if wb is not None:
                        wbuf = wb
                    elif gate:
                        wbuf = load_gatew(m)
                    else:
                        wbuf = load_wbuf(m, ot)
                    nout = NHL if gate else P
                    wstride = 8 if gate else P
                    x4 = xall[:].rearrange("p (b hb f) -> p b hb f", b=4, hb=HB)
                    # resolve chunks to slots; split pairs whose slots are not
                    # adjacent in the ring
                    rchunks = []
                    for (t0, w, nblk) in chunks:
                        b = t0 // SLOC
                        if nblk == 2 and slot_of[b + 1] != slot_of[b] + 1:
                            rchunks.append((slot_of[b], 0, SLOC, 1, t0))
                            rchunks.append((slot_of[b + 1], 0, SLOC, 1, t0 + SLOC))
                        else:
                            rchunks.append((slot_of[b], t0 % SLOC, w, nblk, t0))
                    # chunk-major: each chunk finishes (and frees its x
                    # block) before later chunks; copies drain per chunk
                    for (sl, cl, w, nblk, t0) in rchunks:
                        ps = pbps.tile([P, 512], F32, tag="qkvps", name="qkvps")
                        for hb in range(HB):
                            if nblk == 2:
                                rhs = x4[:, sl:sl + 2, hb, :]
                            else:
                                rhs = x4[:, sl, hb, cl:cl + w]
                            nc.tensor.matmul(
                                ps[:nout, :w],
                                wbuf[:, hb * wstride:hb * wstride + nout], rhs,
                                start=(hb == 0), stop=(hb == HB - 1))
                        nc.vector.tensor_copy(
                            qkvT[ot][:nout, t0:t0 + w], ps[:nout, :w])

                def phase_c(kk):
                    src = qkvT[kk]
                    is_q = kk < NHL
                    sq = pc.tile([P, S], BF, tag="csq", name="csq")
                    nc.gpsimd.tensor_mul(sq[:], src[:], src[:])
                    invrow = pc.tile([1, S], BF, tag="invrow", name="invrow")
                    for ic in range(S // 512):
                        ssp = pcps.tile([1, 512], F32, tag="cps", name="ssp")
                        nc.tensor.matmul(ssp[:], ones_b[:],
                                         sq[:, ic * 512:(ic + 1) * 512],
                                         start=True, stop=True)
                        z = pc.tile([1, 512], F32, tag="cz", name="cz")
                        if is_q:
                            # fold 1/sqrt(D): rsqrt(ssq + D*eps)
                            nc.vector.tensor_scalar_add(z[:], ssp[:], D * EPS)
                        else:
                            nc.vector.tensor_scalar(z[:], ssp[:], 1.0 / D, EPS,
                                                    mybir.AluOpType.mult,
                                                    mybir.AluOpType.add)
                        zr = pc.tile([1, 512], F32, tag="czr", name="czr")
                        nc.vector.reciprocal(zr[:], z[:])
                        nc.scalar.activation(invrow[:, ic * 512:(ic + 1) * 512],
                                             zr[:], AF.Sqrt)
                    invdt = drp.tile([1, S], BF, tag="invd", name="invdt")
                    nc.sync.dma_start(invdt[:], invrow[:])
                    invb = pc.tile([P, S], BF, tag="invb", name="invb")
                    nc.sync.dma_start(invb[:], invdt[0:1, :].to_broadcast([P, S]))
                    sh = pc.tile([P, S], BF, tag="csh", name="csh")
                    nc.sync.dma_start(sh[0:64, :], src[64:128, :])
                    nc.sync.dma_start(sh[64:128, :], src[0:64, :])
                    A = ropes[:, (0 if is_q else 2) * S:(1 if is_q else 3) * S]
                    B = ropes[:, (1 if is_q else 3) * S:(2 if is_q else 4) * S]
                    t1 = pc.tile([P, S], BF, tag="ct1", name="ct1")
                    nc.vector.tensor_mul(t1[:], src[:], A)
                    t2 = pc.tile([P, S], BF, tag="ct2", name="ct2")
                    nc.vector.tensor_mul(t2[:], sh[:], B)
                    nc.vector.tensor_add(t1[:], t1[:], t2[:])
                    nc.vector.tensor_mul(src[:], t1[:], invb[:])  # roped in place

                def phase_d(hh):
                    rk = qkvT[NHL]
                    grow = pcg.tile([1, S], BF, tag="grow", name="grow")
                    nc.sync.dma_start(grow[:], gsd[hh:hh + 1, :])

                    def fin1(isl, esum):
                        # denominator matmul + gate/denominator row; deferred
                        # past the next ic's first j so the PE never waits on
                        # the esum vector chain
                        pden = pcps.tile([1, 512], F32, tag="cps", name="pden")
                        nc.tensor.matmul(pden[:], ones_b[:], esum[:],
                                         start=True, stop=True)
                        rden = pd.tile([1, 512], F32, tag="rden", name="rden")
                        nc.vector.reciprocal(rden[:], pden[:])
                        frow = pd.tile([1, 512], BF, tag="frow", name="frow")
                        nc.vector.tensor_mul(frow[:], rden[:], grow[0:1, isl])
                        facd = drp.tile([1, 512], BF, tag="facd", name="facd")
                        nc.sync.dma_start(facd[:], frow[:])
                        facb = pd.tile([P, 512], BF, tag="facb", name="facb")
                        nc.sync.dma_start(facb[:],
                                          facd[0:1, :].to_broadcast([P, 512]))
                        return facb

                    def fin2(isl, po, facb):
                        oev = pd.tile([P, 512], BF, tag="oev", name="oev")
                        nc.vector.tensor_copy(oev[:], po[:])
                        # qkvT[hh][:, isl] (roped q) is dead after its j-loop
                        nc.vector.tensor_mul(ogt[hh][:, isl], oev[:], facb[:])

                    pend = None
                    for ic in range(S // 512):
                        isl = slice(ic * 512, (ic + 1) * 512)
                        po = pdo.tile([P, 512], F32, tag="po", name="po")
                        esum = pd.tile([P, 512], BF, tag="esum", name="esum")
                        esumo = pd.tile([P, 512], BF, tag="esumo", name="esumo")
                        for j in range(S // P):
                            psc = pds.tile([P, 512], F32, tag="psc", name="psc")
                            nc.tensor.matmul(psc[:], rk[:, j * P:(j + 1) * P],
                                             qkvT[hh][:, isl], start=True, stop=True)
                            es = pes.tile([P, 512], BF, tag="es", name="es")
                            nc.scalar.activation(es[:], psc[:], AF.Exp)
                            nc.tensor.matmul(po[:], v_nat[:, j * P:(j + 1) * P], es[:],
                                             start=(j == 0), stop=(j == S // P - 1))
                            # split exp-sum chains across vector and gpsimd
                            if j == 0:
                                nc.vector.tensor_copy(esum[:], es[:])
                                if pend is not None:
                                    pisl, ppo, pesum = pend
                                    pfacb = fin1(pisl, pesum)
                                    pend = (pisl, ppo, pfacb)
                            elif j == 1:
                                nc.gpsimd.tensor_copy(esumo[:], es[:])
                            elif j % 2 == 0:
                                nc.vector.tensor_add(esum[:], esum[:], es[:])
                            else:
                                nc.gpsimd.tensor_add(esumo[:], esumo[:], es[:])
                        nc.vector.tensor_add(esum[:], esum[:], esumo[:])
                        if pend is not None:
                            pisl, ppo, pfacb = pend
                            fin2(pisl, ppo, pfacb)
                        pend = (isl, po, esum)
                    pisl, ppo, pesum = pend
                    pfacb = fin1(pisl, pesum)
                    fin2(pisl, ppo, pfacb)

                for m in range(2):
                    load_xblks(m)
                    for ot in OT_ORDER:
                        phase_b(m, ot, wb=wpre.pop((m, ot), None))
                    if m == 0:
                        load_consts()
                load_xblks(2)
                for ot in OT_ORDER[:3]:
                    phase_b(2, ot)
                    if ot == 7:
                        # gate logits need the pre-norm scalar: g*inv, sigmoid
                        invb5 = pc.tile([NHL, S], BF, tag="invb5", name="invb5")
                        nc.sync.dma_start(
                            invb5[:],
                            iag_out.rearrange("r o t -> o (r t)")[0:1, :]
                            .to_broadcast([NHL, S]))
                        nc.vector.tensor_mul(qkvT[7][0:NHL, :],
                                             qkvT[7][0:NHL, :], invb5[:])
                        grows = pc.tile([NHL, S], BF, tag="grows", name="grows")
                        nc.scalar.activation(grows[:], qkvT[7][0:NHL, :],
                                             AF.Sigmoid)
                        nc.sync.dma_start(gsd[:], grows[:])
                    elif ot == 6:
                        pass  # v ready; transposed after gates leave qkvT[7]
                    elif ot == 5:
                        # inv columns for v scaling: [t_local, block] layout
                        invsbh = pc.tile([P, S // P], BF, tag="invsbh",
                                         name="invsbh")
                        nc.sync.dma_start(
                            invsbh[:].rearrange("p (r b) -> p r b", r=NCORE),
                            iag_out.rearrange("r o (b p) -> p (r o) b", b=2))
                        invsb = pc.tile([P, S // P], F32, tag="invsb",
                                        name="invsb")
                        nc.scalar.activation(invsb[:], invsbh[:], AF.Copy)
                        for j in range(S // P):
                            tp = pcps.tile([P, P], BF, tag="cps", name="vtp")
                            nc.tensor.transpose(tp[:], qkvT[6][:, j * P:(j + 1) * P],
                                                identb[:])
                            nc.vector.tensor_scalar_mul(
                                v_nat[:, j * P:(j + 1) * P], tp[:],
                                invsb[:, j:j + 1])
                        phase_c(NHL)  # k
                # q heads software-pipelined: B(h) -> C(h) with D(h-1)
                # emitted after C(h) so the PE queue never waits on the
                # rope chain of the head it is about to process
                for h in range(NHL):
                    phase_b(2, h)
                    phase_c(h)
                    if h > 0:
                        phase_d(h - 1)
                phase_d(NHL - 1)
            pbw_cm.__exit__(None, None, None)

            # ---------------- phase E: routed proj + chunked ReduceScatter ---
            with tc.tile_pool(name="phE_w", bufs=2) as pew, \
                 tc.tile_pool(name="phE", bufs=8) as pe, \
                 tc.tile_pool(name="phE_h", bufs=1) as peh, \
                 tc.tile_pool(name="phE_ps", bufs=8, space="PSUM") as peps:
                for ch, (ol0, nol) in enumerate(ECH):
                    cwc = nol * 512
                    holds = {}
                    for m in range(M):
                        lo, hi = off[m], off[m + 1]
                        pw = pew.tile([P, 2 * NHL * 512], BF, tag="pw", name="pw")
                        nc.sync.dma_start(
                            pw[:, :nol * NHL * 512].rearrange(
                                "p (oc hb f) -> p oc hb f", oc=nol, hb=NHL),
                            projw_ext[m, ol0:ol0 + nol].rearrange(
                                "oc p hb f -> p oc hb f"))
                        tc0, tc1 = lo // P, (hi - 1) // P
                        for tcx in range(tc0, tc1 + 1):
                            ystage = pe.tile([P, CW], BF, tag="ystage",
                                             name="ystage")
                            for ol in range(nol):
                                ps = peps.tile([P, 512], F32, tag="yps", name="yps")
                                for hb in range(NHL):
                                    nc.tensor.matmul(
                                        ps[:], ogt[hb][:, tcx * P:(tcx + 1) * P],
                                        pw[:, (ol * NHL + hb) * 512:
                                           (ol * NHL + hb + 1) * 512],
                                        start=(hb == 0), stop=(hb == NHL - 1))
                                nc.vector.tensor_copy(
                                    ystage[:, ol * 512:(ol + 1) * 512], ps[:])
                            if tcx in bnds:
                                bidx = bnds[tcx]
                                if m == bidx:  # lower mod: hold masked partial
                                    hv = peh.tile([P, CW], BF, tag=f"hold{bidx}",
                                                  name=f"hold{bidx}")
                                    nc.vector.tensor_scalar_mul(
                                        hv[:, :cwc], ystage[:, :cwc],
                                        bmask[:, 2 * bidx:2 * bidx + 1])
                                    holds[tcx] = hv
                                else:  # upper mod: merge with inverse mask
                                    hv = holds.pop(tcx)
                                    mg = pe.tile([P, CW], BF, tag="mg", name="mg")
                                    nc.vector.tensor_scalar_mul(
                                        mg[:, :cwc], ystage[:, :cwc],
                                        bmask[:, 2 * bidx + 1:2 * bidx + 2])
                                    yo = pe.tile([P, CW], BF, tag="yo", name="yo")
                                    nc.vector.tensor_add(yo[:, :cwc], hv[:, :cwc],
                                                         mg[:, :cwc])
                                    nc.sync.dma_start(
                                        yh[ch][tcx * P:(tcx + 1) * P, :],
                                        yo[:, :cwc])
                            else:
                                nc.sync.dma_start(
                                    yh[ch][tcx * P:(tcx + 1) * P, :],
                                    ystage[:, :cwc])
                    nc.gpsimd.collective_compute(
                        "ReduceScatter", mybir.AluOpType.add, replica_groups=RG,
                        ins=[yh[ch][:]], outs=[rsh[ch][:]])
                    nc.sync.dma_start(out_ext[:, ol0 * 512:(ol0 + nol) * 512],
                                      rsh[ch][:])

    _split_excess_waits(nc)
    return nc


# ---------------- host-side prep ----------------

def _prep_shards(x, rope_cos, rope_sin, modality_ids, pre_norm, qkv_w, q_norm,
                 k_norm, proj_w, perm, counts):
    """Build the 8 per-core in_maps (host work is index/layout prep only)."""
    bf16 = ml_dtypes.bfloat16
    mods = np.asarray(modality_ids).reshape(S)
    x2 = np.asarray(x).reshape(S, H)[perm]                      # sorted tokens
    cos = np.asarray(rope_cos).reshape(S, D)[perm]
    sin = np.asarray(rope_sin).reshape(S, D)[perm]
    mods_s = mods[perm]
    pn = np.asarray(pre_norm).reshape(M, H)
    qn = np.asarray(q_norm).reshape(M, D)
    kn = np.asarray(k_norm).reshape(M, D)
    qkv3 = np.asarray(qkv_w).reshape(M, QKV_PER_MOD, H)
    proj3 = np.asarray(proj_w).reshape(M, H, H)

    # x^T block layout [block, d, hb, t_local], bf16, replicated per core
    xt = np.ascontiguousarray(
        x2.reshape(NCORE, SLOC, HB, P).transpose(0, 3, 2, 1)).astype(bf16)

    # rope tables [d, t]: roped = q*A + shift64(q)*B   (norm weights folded in)
    def rope_tables(nw_tok):
        A = nw_tok * cos                                        # [S, D]
        Bm = np.empty_like(A)
        nw_sh = np.concatenate([nw_tok[:, 64:], nw_tok[:, :64]], axis=1)
        Bm[:, :64] = -nw_sh[:, :64] * sin[:, :64]
        Bm[:, 64:] = nw_sh[:, 64:] * sin[:, 64:]
        return A.T.astype(bf16), Bm.T.astype(bf16)              # [D, S]

    Aq, Bq = rope_tables(qn[mods_s])
    Ak, Bk = rope_tables(kn[mods_s])
    ropes = np.ascontiguousarray(np.stack([Aq, Bq, Ak, Bk]))    # [4, 128, S]

    # boundary masks for proj chunks
    off = [0, counts[0], counts[0] + counts[1], S]
    bmask = np.zeros((4, P, 1), np.float32)
    for b in (1, 2):
        if off[b] % P != 0:
            tcx = off[b] // P
            toks = tcx * P + np.arange(P)
            bmask[2 * (b - 1), :, 0] = (toks < off[b]).astype(np.float32)
            bmask[2 * (b - 1) + 1, :, 0] = (toks >= off[b]).astype(np.float32)

    in_maps = []
    for c in range(NCORE):
        rq = qkv3[:, c * NHL * D:(c + 1) * NHL * D, :]          # [3, 640, H]
        rk = qkv3[:, Q_DIM + c * D:Q_DIM + (c + 1) * D, :]      # [3, 128, H]
        rv = qkv3[:, Q_DIM + K_DIM + c * D:Q_DIM + K_DIM + (c + 1) * D, :]
        rg = qkv3[:, Q_DIM + 2 * K_DIM + c * NHL:Q_DIM + 2 * K_DIM + (c + 1) * NHL, :]
        Wm = np.concatenate([rq, rk, rv], axis=1)               # [3, 896, H]
        Wm = Wm * pn[:, None, :]                                # fold pre_norm
        WT = Wm.transpose(0, 2, 1)                              # [3, H, 896]
        qkvw = WT.reshape(M, HB, P, 7, P).transpose(0, 3, 2, 1, 4)
        qkvw = np.ascontiguousarray(qkvw).astype(bf16)          # [3,7,128,40,128]
        Gm = np.concatenate(
            [rg, np.zeros((M, 3, H), np.float32)], axis=1) * pn[:, None, :]
        gatew = Gm.transpose(0, 2, 1).reshape(M, HB, P, 8).transpose(0, 2, 1, 3)
        gatew = np.ascontiguousarray(gatew).astype(bf16)        # [3,128,40,8]

        PT = proj3[:, :, c * NHL * D:(c + 1) * NHL * D].transpose(0, 2, 1)
        projw = PT.reshape(M, NHL, P, NCH, 512).transpose(0, 3, 2, 1, 4)
        projw = np.ascontiguousarray(projw).astype(bf16)        # [3,10,128,5,512]

        in_maps.append({
            "xs": np.ascontiguousarray(x2[c * SLOC:(c + 1) * SLOC]).astype(bf16),
            "xt": xt,
            "qkvw": qkvw,
            "gatew": gatew,
            "projw": projw,
            "ropes": ropes,
            "bmask": bmask,
        })
    return in_maps


_CACHE = {}


def _get_module(counts):
    key = tuple(counts)
    if key not in _CACHE:
        _CACHE[key] = build_module(counts)
    return _CACHE[key]


def kernel(x, rope_cos, rope_sin, modality_ids, pre_norm, qkv_w, q_norm,
           k_norm, proj_w):
    mods = np.asarray(modality_ids).reshape(S)
    perm = np.argsort(mods, kind="stable")
    counts = [int((mods == m).sum()) for m in range(M)]
    assert min(counts) >= P, "modality segments must span at least one chunk"
    nc = _get_module(counts)
    in_maps = _prep_shards(x, rope_cos, rope_sin, modality_ids, pre_norm,
                           qkv_w, q_norm, k_norm, proj_w, perm, counts)
    res = run_bass_kernel_spmd(nc, in_maps, list(range(NCORE)), trace=False)
    y_sorted = np.empty((S, H), np.float32)
    for c in range(NCORE):
        y_sorted[c * SLOC:(c + 1) * SLOC] = res.results[c]["out"]
    y = np.empty_like(y_sorted)
    y[perm] = y_sorted
    return y[None]


from concourse.bass_utils import run_bass_kernel_spmd  # noqa: E402
